# revision 11
# baseline (speedup 1.0000x reference)
"""GCN-Attention kernel for Trainium2, data-parallel over 8 NeuronCores.

Reference computation (per image b of 64, category c of 100):
  full = concat(image_features, bbox)                    [N, 2052]
  x[b,c,:] = sum_{boxes n in bucket(b,c), slot<3} lin_w[slot]*full[n] + lin_b
  support  = x @ gc_w                                    [B, 100, 2048]
  gcn      = leaky_relu((X + adj) @ support + gc_b)
  out[b]   = global_features[b] @ gcn[b]                 [B, 2048]

Matmul associativity moves the (tiny) adjacency product left of the big
GEMM: (X+adj) @ (x @ gc_w) = ((X+adj) @ x) @ gc_w.  The host resolves the
occurrence-slot scatter into x and pre-multiplies y = (X+adj) @ x in f32
(~5% of total FLOPs), so the device work per core collapses to ONE
M-packed GEMM + pointwise + attention rows:

  phase A: Z = y_flat @ gc_w with y_flat [800, 2052] (8 images x 100
           categories packed along M into 7 tiles of <=128).  Emitted as
           4 N-passes (512 cols each) x 17 K-chunks x 7 M-tiles with the
           K loop outer, so pass 0 consumes gc_w chunk k only ~1.5us x k
           into the kernel and the PE starts ~2us in instead of waiting
           for the full weight load.  The 17th K-chunk (5 rows) carries
           the 4 bbox features and the gc_b bias row.
  phase B: leaky-relu drains PSUM -> bf16 SBUF, alternating between the
           scalar engine (activation) and the vector engine
           (scalar_tensor_tensor max(x, 0.01x)) so the 7 drains of a pass
           clear within the PSUM-bank reuse window of the next pass.
  phase C: attention rows as [tw,8]^T @ [tw,512] matmuls: per M-tile one
           stationary [128, 8] matrix holding each image's
           global-feature weights at that image's packed rows (zeros
           elsewhere), accumulated over the 7 M-tiles into one PSUM bank.
           Interleaved into the next pass's matmul stream (dep-anchored
           so the scheduler cannot hoist them ahead of the covering
           drain).  The last pass runs M-pair-grouped so its attention
           rows overlap the pass tail instead of serializing after it.

PSUM: 7 pass accumulators + 1 attention bank = exactly 8 banks.
"""
import time

import ml_dtypes
import numpy as np

import concourse.bacc as bacc
import concourse.mybir as mybir
import concourse.tile as tile
from concourse import bass_utils

B = 64
C = 100
LOOP = 3
FEAT = 2052
OUT = 2048
NCORES = 8
BPC = B // NCORES  # images per core
MROWS = BPC * C    # 800 packed M rows per core
TM = 127           # M-tile rows: 127 (not 128) keeps FWL off so LDWEIGHTS
                   # (1 XBUS) overlaps the running matmul's rhs stream
NMT = 7            # M tiles of <=TM
NKT = 17           # K chunks: 16 x 128 + 1 x 5 (bbox + bias)
NNCH = 4           # N passes of 512

f32 = mybir.dt.float32
bf16 = mybir.dt.bfloat16
np_bf16 = ml_dtypes.bfloat16

_programs: dict = {}
last_results = None  # BassKernelResults of the most recent run (for harnesses)


def _occ_slots(key):
    """Occurrence index among equal-valued keys, stable order (matches jax ref)."""
    n = key.shape[0]
    order = np.argsort(key, kind="stable")
    sk = key[order]
    idx = np.arange(n)
    is_new = np.concatenate([[True], sk[1:] != sk[:-1]]) if n else np.zeros(0, bool)
    run_start = np.maximum.accumulate(np.where(is_new, idx, 0))
    pos = idx - run_start
    slots = np.zeros(n, np.int64)
    slots[order] = pos
    return slots


def _mw(m):
    return TM if m < NMT - 1 else MROWS - TM * (NMT - 1)


def _kw(k):
    return 128 if k < NKT - 1 else 5


def _build():
    nc = bacc.Bacc("TRN2", target_bir_lowering=False, debug=False,
                   num_devices=NCORES)

    # partition-major dram layouts so chunk DMAs merge into contiguous spans
    xt_d = nc.dram_tensor("xt", [128, NKT * MROWS], bf16, kind="ExternalInput").ap()
    gcwp_d = nc.dram_tensor("gcwp", [NNCH, 128, NKT * 512], bf16,
                            kind="ExternalInput").ap()
    gtp_d = nc.dram_tensor("gtp", [128, NMT * BPC], bf16, kind="ExternalInput").ap()
    out_d = nc.dram_tensor("out", [BPC, OUT], f32, kind="ExternalOutput").ap()

    with tile.TileContext(nc) as tc:
        with tc.tile_pool(name="const", bufs=1) as cpool, \
             tc.tile_pool(name="sb", bufs=1) as pool, \
             tc.tile_pool(name="ps", bufs=1, space="PSUM") as psp:

            # gc_w resident: pass 0's chunks land individually (consumed at
            # ~1.5us x k into the kernel), passes 1-3 as one DMA each; the
            # gpsimd queue keeps sync free for xt.  Few large DMAs keep the
            # framework's semaphore count (and its end-of-kernel semaphore
            # reset, ~115ns each) down.
            gcw_sb = cpool.tile([128, NNCH * NKT * 512], bf16, tag="gcw")
            for k in range(NKT):
                kw = _kw(k)
                nc.gpsimd.dma_start(gcw_sb[0:kw, k * 512:(k + 1) * 512],
                                    gcwp_d[0][0:kw, k * 512:(k + 1) * 512])
            for nch in range(1, NNCH):
                o = nch * NKT * 512
                nc.gpsimd.dma_start(gcw_sb[0:128, o:o + NKT * 512],
                                    gcwp_d[nch])
            # y^T chunks on the sync queue: k-singles while the PE chews
            # ~1.5us per chunk, then the remainder in one DMA
            xt_sb = cpool.tile([128, NKT * MROWS], bf16, tag="xt")
            XT_SINGLE = 10
            for k in range(XT_SINGLE):
                nc.sync.dma_start(xt_sb[0:128, k * MROWS:(k + 1) * MROWS],
                                  xt_d[0:128, k * MROWS:(k + 1) * MROWS])
            nc.sync.dma_start(
                xt_sb[0:128, XT_SINGLE * MROWS:NKT * MROWS],
                xt_d[0:128, XT_SINGLE * MROWS:NKT * MROWS])
            # per-M-tile zero-padded attention weights
            gtp_sb = cpool.tile([128, NMT * BPC], bf16, tag="gtp")
            nc.scalar.dma_start(gtp_sb[:], gtp_d[:])
            # force the Lrelu spline-table load (~2.7us) under the DMA head
            warm = pool.tile([1, BPC], f32, tag="warm", bufs=1)
            nc.scalar.activation(warm[:], gtp_sb[0:1, 0:BPC],
                                 mybir.ActivationFunctionType.Lrelu, alpha=0.01)

            gcn = {}       # (nch, m) -> drained bf16 tile
            ps4 = {}       # nch -> attention PSUM tile
            ps4_mm = {}    # nch -> number of attention matmuls emitted

            def pass_mm(ps_tiles, nch, k, m):
                kw, mw = _kw(k), _mw(m)
                ko = k * MROWS + TM * m
                wo = (nch * NKT + k) * 512
                return nc.tensor.matmul(
                    ps_tiles[m][0:mw, 0:512],
                    xt_sb[0:kw, ko:ko + mw],
                    gcw_sb[0:kw, wo:wo + 512],
                    start=(k == 0), stop=(k == NKT - 1),
                )

            def drain(ps_tiles, nch, m, eng):
                # leaky-relu PSUM -> bf16; split across engines so the drains
                # of a pass clear inside the next pass's bank-reuse window
                mw = _mw(m)
                g = pool.tile([128, 512], bf16, tag="gcn", bufs=14,
                              name=f"gcn_{nch}_{m}")
                src = ps_tiles[m][0:mw, 0:512]
                if eng == "s":
                    nc.scalar.activation(g[0:mw, :], src,
                                         mybir.ActivationFunctionType.Lrelu,
                                         alpha=0.01)
                else:
                    tmp = pool.tile([128, 512], bf16, tag="lrt", bufs=2,
                                    name=f"lrt_{nch}_{m}")
                    nc.vector.tensor_scalar_mul(tmp[0:mw, :], src, 0.01)
                    nc.vector.tensor_max(g[0:mw, :], src, tmp[0:mw, :])
                gcn[(nch, m)] = g

            def ph4(nch, m, anchor=None, ps_tag="ps4"):
                # attention row partials: [tw, 8]^T @ [tw, 512] accumulated
                # over the 7 M-tiles into one PSUM bank (rows 0:8)
                if nch not in ps4:
                    ps4[nch] = psp.tile([128, 512], f32, tag=ps_tag, bufs=1,
                                        name=f"ps4_{nch}")
                    ps4_mm[nch] = 0
                tw = _mw(m)
                mi = nc.tensor.matmul(
                    ps4[nch][0:BPC, 0:512],
                    gtp_sb[0:tw, m * BPC:(m + 1) * BPC],
                    gcn[(nch, m)][0:tw, 0:512],
                    start=(ps4_mm[nch] == 0), stop=(ps4_mm[nch] == NMT - 1),
                )
                ps4_mm[nch] += 1
                if anchor is not None:
                    tile.add_dep_helper(mi.ins, anchor.ins, sync=False,
                                        reason="defer ph4 behind pass")

            def ps4_out(nch):
                ost = pool.tile([BPC, 512], f32, tag="ost", bufs=2,
                                name=f"ost_{nch}")
                nc.vector.tensor_copy(ost[:], ps4[nch][0:BPC, 0:512])
                nc.sync.dma_start(out_d[0:BPC, nch * 512:(nch + 1) * 512],
                                  ost[:])

            # pass 0: K outer, M inner — consumes weight chunk k only ~1.5us
            # x k into the kernel, so the PE is never DMA-gated at the head.
            # Drains emitted bank-0/1 first (next pass needs those first).
            nch = 0
            ps_tiles = [psp.tile([128, 512], f32, tag=f"ps{m}", bufs=1,
                                 name=f"ps_{nch}_{m}") for m in range(NMT)]
            for k in range(NKT):
                for m in range(NMT):
                    pass_mm(ps_tiles, nch, k, m)
            for m, eng in ((0, "s"), (1, "s"), (2, "v"), (3, "v"),
                           (4, "s"), (5, "v"), (6, "s")):
                drain(ps_tiles, nch, m, eng)

            # passes 1..3: M-grouped (pairs + final triple, K interleaved
            # inside a group to keep same-bank accumulating matmuls apart).
            # Tile completions stagger through the pass, so drains run
            # mid-pass and every bank-reuse deadline has ~7us of slack.
            # The previous pass's attention rows interleave into groups 0-1;
            # pass 3's own interleave into its final group and tail.
            groups = [(0, 1), (2, 3), (4, 5, 6)]
            for nch in range(1, NNCH):
                ps_tiles = [psp.tile([128, 512], f32, tag=f"ps{m}", bufs=1,
                                     name=f"ps_{nch}_{m}") for m in range(NMT)]
                for gi, grp in enumerate(groups):
                    for k in range(NKT):
                        last = None
                        for m in grp:
                            last = pass_mm(ps_tiles, nch, k, m)
                        if gi == 0 and k in (4, 8, 12, 16):
                            ph4(nch - 1, (k - 4) // 4, anchor=last)
                        if gi == 1 and k in (4, 9, 14):
                            ph4(nch - 1, 4 + (k - 4) // 5, anchor=last)
                        if nch == NNCH - 1 and gi == 2 and k in (2, 6, 10, 14):
                            ph4(nch, (k - 2) // 4, anchor=last)
                    if gi == 1:
                        ps4_out(nch - 1)
                    for i, m in enumerate(grp):
                        drain(ps_tiles, nch, m, "s" if i % 2 == 0 else "v")
            nch = NNCH - 1
            for m in (4, 5, 6):
                ph4(nch, m)
            ps4_out(nch)

    nc.compile()
    return nc


def _get_program():
    if "main" not in _programs:
        _programs["main"] = _build()
    return _programs["main"]


def kernel(**inputs) -> np.ndarray:
    global last_results

    imf = np.asarray(inputs["image_features"], np.float32)
    bbox = np.asarray(inputs["bbox_list"], np.float32)
    gf = np.asarray(inputs["global_features"], np.float32)
    adj = np.asarray(inputs["adj"], np.float32)
    X = np.asarray(inputs["X"], np.float32)
    lin_w = np.asarray(inputs["lin_w"], np.float32)
    lin_b = np.float32(np.asarray(inputs["lin_b"]))
    gc_w = np.ascontiguousarray(np.asarray(inputs["gc_w"], np.float32))
    gc_b = np.asarray(inputs["gc_b"], np.float32)
    label = np.asarray(inputs["label_list"]).astype(np.int64)
    batch = np.asarray(inputs["batch"]).astype(np.int64)

    n = imf.shape[0]
    full = np.concatenate([imf, bbox], axis=1)

    # scatter bookkeeping, matching jax semantics: slots by stable order of
    # key=batch*C+(label-1); negative cats wrap, slot>=LOOP / far-oob dropped
    cat = label - 1
    key = batch * C + cat
    slots = _occ_slots(key)
    valid = (slots < LOOP) & (cat >= -C) & (cat < C)
    wvals = np.where(valid, lin_w[np.clip(slots, 0, LOOP - 1)], 0.0).astype(np.float32)
    cidx = np.mod(cat, C).astype(np.int64)

    # host scatter-sum: S[b,c,:] = sum of lin_w[slot]*full over the <=LOOP
    # boxes of bucket (b,c); slots are unique per bucket so per-slot
    # fancy-index adds have no collisions
    S = np.zeros((B, C, FEAT), np.float32)
    bok = valid & (batch >= -B) & (batch < B)
    bmod = np.mod(batch, B)
    for s in range(LOOP):
        sel = bok & (slots == s)
        if np.any(sel):
            S[bmod[sel], cidx[sel]] += wvals[sel, None] * full[sel]

    # pre-multiply the adjacency: y = (X + adj) @ (S + lin_b), f32 exact
    newadj = X[None, :, :] + adj                       # [B, C, C]
    y = np.matmul(newadj, S + lin_b)                   # [B, C, FEAT]

    # gc_w packed per (N-pass, K-chunk); 17th chunk = bbox rows + gc_b row;
    # stored partition-major [NNCH, 128, NKT*512] so pass DMAs are contiguous
    gcwp = np.zeros((NNCH, NKT, 128, 512), np.float32)
    gcwp[:, 0:16] = gc_w[0:2048].reshape(16, 128, NNCH, 512).transpose(2, 0, 1, 3)
    gcwp[:, 16, 0:4] = gc_w[2048:FEAT].reshape(4, NNCH, 512).transpose(1, 0, 2)
    gcwp[:, 16, 4] = gc_b.reshape(NNCH, 512)
    gcwp = np.ascontiguousarray(gcwp.transpose(0, 2, 1, 3)).reshape(
        NNCH, 128, NKT * 512).astype(np_bf16)

    in_maps = []
    for core in range(NCORES):
        imgs = slice(core * BPC, (core + 1) * BPC)
        yf = y[imgs].reshape(MROWS, FEAT)
        xt = np.zeros((NKT, 128, MROWS), np.float32)
        xt[0:16] = np.ascontiguousarray(yf[:, 0:2048].T).reshape(16, 128, MROWS)
        xt[16, 0:4] = yf[:, 2048:FEAT].T
        xt[16, 4] = 1.0
        xt = np.ascontiguousarray(xt.transpose(1, 0, 2)).reshape(
            128, NKT * MROWS)
        # zero-padded attention weights: row r of M-tile m = packed row
        # R=TM*m+r = (image R//100, category R%100) -> gf value in column
        # R//100, zero elsewhere
        gtp = np.zeros((NMT, 128, BPC), np.float32)
        R = np.arange(MROWS)
        gtp[R // TM, R % TM, R // C] = gf[imgs][R // C, R % C]
        gtp = np.ascontiguousarray(gtp.transpose(1, 0, 2)).reshape(
            128, NMT * BPC)
        in_maps.append(dict(
            xt=xt.astype(np_bf16), gcwp=gcwp, gtp=gtp.astype(np_bf16)))

    nc = _get_program()
    res = None
    for attempt in range(4):
        try:
            res = bass_utils.run_bass_kernel_spmd(
                nc, in_maps, core_ids=list(range(NCORES)))
            break
        except Exception:
            if attempt == 3:
                raise
            time.sleep(3 * (attempt + 1))  # transient NRT exec-unit errors
    last_results = res
    return np.concatenate([res.results[i]["out"] for i in range(NCORES)], axis=0)


# revision 20
# speedup vs baseline: 1.0050x; 1.0050x over previous
"""GCN-Attention kernel for Trainium2, data-parallel over 8 NeuronCores.

Reference computation (per image b of 64, category c of 100):
  full = concat(image_features, bbox)                    [N, 2052]
  x[b,c,:] = sum_{boxes n in bucket(b,c), slot<3} lin_w[slot]*full[n] + lin_b
  support  = x @ gc_w                                    [B, 100, 2048]
  gcn      = leaky_relu((X + adj) @ support + gc_b)
  out[b]   = global_features[b] @ gcn[b]                 [B, 2048]

Matmul associativity moves the (tiny) adjacency product left of the big
GEMM: (X+adj) @ (x @ gc_w) = ((X+adj) @ x) @ gc_w.  The host resolves the
occurrence-slot scatter into x and pre-multiplies y = (X+adj) @ x in f32
(~5% of total FLOPs), so the device work per core collapses to ONE
M-packed GEMM + pointwise + attention rows:

  phase A: Z = y_flat @ gc_w with y_flat [800, 2052] (8 images x 100
           categories packed along M into 7 tiles of <=128).  Emitted as
           4 N-passes (512 cols each) x 17 K-chunks x 7 M-tiles with the
           K loop outer, so pass 0 consumes gc_w chunk k only ~1.5us x k
           into the kernel and the PE starts ~2us in instead of waiting
           for the full weight load.  The 17th K-chunk (5 rows) carries
           the 4 bbox features and the gc_b bias row.
  phase B: leaky-relu drains PSUM -> bf16 SBUF, alternating between the
           scalar engine (activation) and the vector engine
           (scalar_tensor_tensor max(x, 0.01x)) so the 7 drains of a pass
           clear within the PSUM-bank reuse window of the next pass.
  phase C: attention rows as [tw,8]^T @ [tw,512] matmuls: per M-tile one
           stationary [128, 8] matrix holding each image's
           global-feature weights at that image's packed rows (zeros
           elsewhere), accumulated over the 7 M-tiles into one PSUM bank.
           Interleaved into the next pass's matmul stream (dep-anchored
           so the scheduler cannot hoist them ahead of the covering
           drain).  The last pass runs M-pair-grouped so its attention
           rows overlap the pass tail instead of serializing after it.

PSUM: 7 pass accumulators + 1 attention bank = exactly 8 banks.
"""
import time

import ml_dtypes
import numpy as np

import concourse.bacc as bacc
import concourse.mybir as mybir
import concourse.tile as tile
from concourse import bass_utils

B = 64
C = 100
LOOP = 3
FEAT = 2052
OUT = 2048
NCORES = 8
BPC = B // NCORES  # images per core
MROWS = BPC * C    # 800 packed M rows per core
TM = 128           # M-tile rows (128 enables the fast weight-load path)
NMT = 7            # M tiles of <=TM
NKT = 17           # K chunks: 16 x 128 + 1 x 5 (bbox + bias)
NNCH = 4           # N passes of 512

f32 = mybir.dt.float32
bf16 = mybir.dt.bfloat16
np_bf16 = ml_dtypes.bfloat16

_programs: dict = {}
last_results = None  # BassKernelResults of the most recent run (for harnesses)


def _occ_slots(key):
    """Occurrence index among equal-valued keys, stable order (matches jax ref)."""
    n = key.shape[0]
    order = np.argsort(key, kind="stable")
    sk = key[order]
    idx = np.arange(n)
    is_new = np.concatenate([[True], sk[1:] != sk[:-1]]) if n else np.zeros(0, bool)
    run_start = np.maximum.accumulate(np.where(is_new, idx, 0))
    pos = idx - run_start
    slots = np.zeros(n, np.int64)
    slots[order] = pos
    return slots


def _mw(m):
    return TM if m < NMT - 1 else MROWS - TM * (NMT - 1)


def _kw(k):
    return 128 if k < NKT - 1 else 5


def _build():
    nc = bacc.Bacc("TRN2", target_bir_lowering=False, debug=False,
                   num_devices=NCORES)

    xt_d = nc.dram_tensor("xt", [NKT, 128, MROWS], bf16, kind="ExternalInput").ap()
    gcwp_d = nc.dram_tensor("gcwp", [NNCH, NKT, 128, 512], bf16,
                            kind="ExternalInput").ap()
    gtp_d = nc.dram_tensor("gtp", [NMT, 128, BPC], bf16, kind="ExternalInput").ap()
    out_d = nc.dram_tensor("out", [BPC, OUT], f32, kind="ExternalOutput").ap()

    with tile.TileContext(nc) as tc:
        with tc.tile_pool(name="const", bufs=1) as cpool, \
             tc.tile_pool(name="sb", bufs=1) as pool, \
             tc.tile_pool(name="ps", bufs=1, space="PSUM") as psp:

            # gc_w resident: 68 chunk DMAs in phase-major order so phase 1's
            # chunks land first; gpsimd queue keeps sync free for xt
            gcw_sb = cpool.tile([128, NNCH * NKT * 512], bf16, tag="gcw")
            for nch in range(NNCH):
                for k in range(NKT):
                    kw = _kw(k)
                    o = (nch * NKT + k) * 512
                    nc.gpsimd.dma_start(gcw_sb[0:kw, o:o + 512],
                                        gcwp_d[nch][k][0:kw, :])
            # y^T chunks, k-major on the sync queue (consumed at ~1.5us/chunk)
            xt_sb = cpool.tile([128, NKT * MROWS], bf16, tag="xt")
            for k in range(NKT):
                kw = _kw(k)
                nc.sync.dma_start(xt_sb[0:kw, k * MROWS:(k + 1) * MROWS],
                                  xt_d[k][0:kw, :])
            # per-M-tile zero-padded attention weights
            gtp_sb = cpool.tile([128, NMT * BPC], bf16, tag="gtp")
            for m in range(NMT):
                nc.scalar.dma_start(gtp_sb[0:128, m * BPC:(m + 1) * BPC],
                                    gtp_d[m])
            # force the Lrelu spline-table load (~2.7us) under the DMA head
            warm = pool.tile([1, BPC], f32, tag="warm", bufs=1)
            nc.scalar.activation(warm[:], gtp_sb[0:1, 0:BPC],
                                 mybir.ActivationFunctionType.Lrelu, alpha=0.01)

            gcn = {}       # (nch, m) -> drained bf16 tile
            ps4 = {}       # nch -> attention PSUM tile
            ps4_mm = {}    # nch -> number of attention matmuls emitted

            def pass_mm(ps_tiles, nch, k, m):
                kw, mw = _kw(k), _mw(m)
                ko = k * MROWS + TM * m
                wo = (nch * NKT + k) * 512
                return nc.tensor.matmul(
                    ps_tiles[m][0:mw, 0:512],
                    xt_sb[0:kw, ko:ko + mw],
                    gcw_sb[0:kw, wo:wo + 512],
                    start=(k == 0), stop=(k == NKT - 1),
                )

            def drain(pst, nch, m, eng):
                # leaky-relu PSUM -> bf16; split across engines so the drains
                # of a tile clear inside the bank-reuse window
                mw = _mw(m)
                g = pool.tile([128, 512], bf16, tag="gcn", bufs=14,
                              name=f"gcn_{nch}_{m}")
                src = pst[0:mw, 0:512]
                if eng == "s":
                    nc.scalar.activation(g[0:mw, :], src,
                                         mybir.ActivationFunctionType.Lrelu,
                                         alpha=0.01)
                else:
                    tmp = pool.tile([128, 512], bf16, tag="lrt", bufs=2,
                                    name=f"lrt_{nch}_{m}")
                    nc.vector.tensor_scalar_mul(tmp[0:mw, :], src, 0.01)
                    nc.vector.tensor_max(g[0:mw, :], src, tmp[0:mw, :])
                gcn[(nch, m)] = g

            # PE-stream order is frozen with nosync deps: phase 2 reuses the
            # loaded stationary across 3 matmuls (ldweights=False), so no PE
            # instruction may be reordered across a weight load.
            prev_pe = [None]

            def pe_chain(mi):
                if prev_pe[0] is not None:
                    tile.add_dep_helper(mi.ins, prev_pe[0].ins, sync=False,
                                        reason="freeze PE order")
                prev_pe[0] = mi
                return mi

            def ph4(nch, m, ps_tag):
                # attention row partials: [tw, 8]^T @ [tw, 512] accumulated
                # over the 7 M-tiles into one PSUM bank (rows 0:8)
                if nch not in ps4:
                    ps4[nch] = psp.tile([128, 512], f32, tag=ps_tag, bufs=1,
                                        name=f"ps4_{nch}")
                    ps4_mm[nch] = 0
                tw = _mw(m)
                mi = nc.tensor.matmul(
                    ps4[nch][0:BPC, 0:512],
                    gtp_sb[0:tw, m * BPC:(m + 1) * BPC],
                    gcn[(nch, m)][0:tw, 0:512],
                    start=(ps4_mm[nch] == 0), stop=(ps4_mm[nch] == NMT - 1),
                )
                ps4_mm[nch] += 1
                pe_chain(mi)

            def ps4_out(nch):
                ost = pool.tile([BPC, 512], f32, tag="ost", bufs=2,
                                name=f"ost_{nch}")
                nc.vector.tensor_copy(ost[:], ps4[nch][0:BPC, 0:512])
                nc.sync.dma_start(out_d[0:BPC, nch * 512:(nch + 1) * 512],
                                  ost[:])

            # phase 1 (N-cols 0:512): K outer, M inner — consumes weight
            # chunk k only ~1.5us x k into the kernel, so the PE is never
            # DMA-gated at the head.  Its 7 accumulators use the pp ring +
            # three of the attention banks (reused by phase 2 afterwards).
            P1TAG = ["pp", "pp", "pp", "pp", "ps4a", "ps4b", "ps4c"]
            ps_tiles = [psp.tile([128, 512], f32, tag=P1TAG[m],
                                 bufs=(4 if P1TAG[m] == "pp" else 1),
                                 name=f"p1_{m}") for m in range(NMT)]
            for k in range(NKT):
                for m in range(NMT):
                    pe_chain(pass_mm(ps_tiles, 0, k, m))
            # drain order matters: phase 2's tile 0 reuses pp banks 0..2
            # first; the attention banks (m 4..6) are needed only ~3us+ in
            for m, eng in ((0, "s"), (1, "v"), (2, "s"), (3, "v"),
                           (4, "s"), (5, "v"), (6, "s")):
                drain(ps_tiles[m], 0, m, eng)

            # phase 2 (N-cols 512:2048): M outer, K inner; one weight load
            # serves the three N-chunk matmuls (ldweights=False on 2nd/3rd).
            # Attention rows interleave per tile; accumulators live on the
            # four ps4 banks.
            for m in range(NMT):
                pps = [psp.tile([128, 512], f32, tag="pp", bufs=4,
                                name=f"p2_{m}_{j}") for j in range(3)]
                for k in range(NKT):
                    kw, mw = _kw(k), _mw(m)
                    ko = k * MROWS + TM * m
                    for j in range(3):
                        wo = ((j + 1) * NKT + k) * 512
                        mi = nc.tensor.matmul(
                            pps[j][0:mw, 0:512],
                            xt_sb[0:kw, ko:ko + mw],
                            gcw_sb[0:kw, wo:wo + 512],
                            start=(k == 0), stop=(k == NKT - 1),
                        )
                        if j > 0:
                            mi.ins.ldweights = False
                        pe_chain(mi)
                    if m == 0 and k in (4, 6, 8, 10, 12, 14, 16):
                        ph4(0, (k - 4) // 2, "ps4a")
                    if m >= 1 and k == 4:
                        ph4(1, m - 1, "ps4b")
                    if m >= 1 and k == 8:
                        ph4(2, m - 1, "ps4c")
                    if m >= 1 and k == 12:
                        ph4(3, m - 1, "ps4d")
                if m == 1:
                    ps4_out(0)
                drain(pps[0], 1, m, "s")
                drain(pps[1], 2, m, "v")
                drain(pps[2], 3, m, "s")
            for nch in range(1, NNCH):
                ph4(nch, NMT - 1, ("ps4b", "ps4c", "ps4d")[nch - 1])
                ps4_out(nch)

    nc.compile()
    return nc


def _get_program():
    if "main" not in _programs:
        _programs["main"] = _build()
    return _programs["main"]


def kernel(**inputs) -> np.ndarray:
    global last_results

    imf = np.asarray(inputs["image_features"], np.float32)
    bbox = np.asarray(inputs["bbox_list"], np.float32)
    gf = np.asarray(inputs["global_features"], np.float32)
    adj = np.asarray(inputs["adj"], np.float32)
    X = np.asarray(inputs["X"], np.float32)
    lin_w = np.asarray(inputs["lin_w"], np.float32)
    lin_b = np.float32(np.asarray(inputs["lin_b"]))
    gc_w = np.ascontiguousarray(np.asarray(inputs["gc_w"], np.float32))
    gc_b = np.asarray(inputs["gc_b"], np.float32)
    label = np.asarray(inputs["label_list"]).astype(np.int64)
    batch = np.asarray(inputs["batch"]).astype(np.int64)

    n = imf.shape[0]
    full = np.concatenate([imf, bbox], axis=1)

    # scatter bookkeeping, matching jax semantics: slots by stable order of
    # key=batch*C+(label-1); negative cats wrap, slot>=LOOP / far-oob dropped
    cat = label - 1
    key = batch * C + cat
    slots = _occ_slots(key)
    valid = (slots < LOOP) & (cat >= -C) & (cat < C)
    wvals = np.where(valid, lin_w[np.clip(slots, 0, LOOP - 1)], 0.0).astype(np.float32)
    cidx = np.mod(cat, C).astype(np.int64)

    # host scatter-sum: S[b,c,:] = sum of lin_w[slot]*full over the <=LOOP
    # boxes of bucket (b,c); slots are unique per bucket so per-slot
    # fancy-index adds have no collisions
    S = np.zeros((B, C, FEAT), np.float32)
    bok = valid & (batch >= -B) & (batch < B)
    bmod = np.mod(batch, B)
    for s in range(LOOP):
        sel = bok & (slots == s)
        if np.any(sel):
            S[bmod[sel], cidx[sel]] += wvals[sel, None] * full[sel]

    # pre-multiply the adjacency: y = (X + adj) @ (S + lin_b), f32 exact
    newadj = X[None, :, :] + adj                       # [B, C, C]
    y = np.matmul(newadj, S + lin_b)                   # [B, C, FEAT]

    # gc_w packed per (N-chunk, K-chunk); 17th chunk = bbox rows + gc_b row
    gcwp = np.zeros((NNCH, NKT, 128, 512), np.float32)
    gcwp[:, 0:16] = gc_w[0:2048].reshape(16, 128, NNCH, 512).transpose(2, 0, 1, 3)
    gcwp[:, 16, 0:4] = gc_w[2048:FEAT].reshape(4, NNCH, 512).transpose(1, 0, 2)
    gcwp[:, 16, 4] = gc_b.reshape(NNCH, 512)
    gcwp = gcwp.astype(np_bf16)

    in_maps = []
    for core in range(NCORES):
        imgs = slice(core * BPC, (core + 1) * BPC)
        yf = y[imgs].reshape(MROWS, FEAT)
        xt = np.zeros((NKT, 128, MROWS), np.float32)
        xt[0:16] = np.ascontiguousarray(yf[:, 0:2048].T).reshape(16, 128, MROWS)
        xt[16, 0:4] = yf[:, 2048:FEAT].T
        xt[16, 4] = 1.0
        # zero-padded attention weights: row r of M-tile m = packed row
        # R=TM*m+r = (image R//100, category R%100) -> gf value in column
        # R//100, zero elsewhere
        gtp = np.zeros((NMT, 128, BPC), np.float32)
        R = np.arange(MROWS)
        gtp[R // TM, R % TM, R // C] = gf[imgs][R // C, R % C]
        in_maps.append(dict(
            xt=xt.astype(np_bf16), gcwp=gcwp, gtp=gtp.astype(np_bf16)))

    nc = _get_program()
    res = None
    for attempt in range(4):
        try:
            res = bass_utils.run_bass_kernel_spmd(
                nc, in_maps, core_ids=list(range(NCORES)))
            break
        except Exception:
            if attempt == 3:
                raise
            time.sleep(3 * (attempt + 1))  # transient NRT exec-unit errors
    last_results = res
    return np.concatenate([res.results[i]["out"] for i in range(NCORES)], axis=0)


# revision 22
# speedup vs baseline: 1.0264x; 1.0214x over previous
"""GCN-Attention kernel for Trainium2, data-parallel over 8 NeuronCores.

Reference computation (per image b of 64, category c of 100):
  full = concat(image_features, bbox)                    [N, 2052]
  x[b,c,:] = sum_{boxes n in bucket(b,c), slot<3} lin_w[slot]*full[n] + lin_b
  support  = x @ gc_w                                    [B, 100, 2048]
  gcn      = leaky_relu((X + adj) @ support + gc_b)
  out[b]   = global_features[b] @ gcn[b]                 [B, 2048]

Matmul associativity moves the (tiny) adjacency product left of the big
GEMM: (X+adj) @ (x @ gc_w) = ((X+adj) @ x) @ gc_w.  The host resolves the
occurrence-slot scatter into x and pre-multiplies y = (X+adj) @ x in f32
(~5% of total FLOPs), so the device work per core collapses to ONE
M-packed GEMM + pointwise + attention rows:

  phase A: Z = y_flat @ gc_w with y_flat [800, 2052] (8 images x 100
           categories packed along M into 7 tiles of <=128).  Emitted as
           4 N-passes (512 cols each) x 17 K-chunks x 7 M-tiles with the
           K loop outer, so pass 0 consumes gc_w chunk k only ~1.5us x k
           into the kernel and the PE starts ~2us in instead of waiting
           for the full weight load.  The 17th K-chunk (5 rows) carries
           the 4 bbox features and the gc_b bias row.
  phase B: leaky-relu drains PSUM -> bf16 SBUF, alternating between the
           scalar engine (activation) and the vector engine
           (scalar_tensor_tensor max(x, 0.01x)) so the 7 drains of a pass
           clear within the PSUM-bank reuse window of the next pass.
  phase C: attention rows as [tw,8]^T @ [tw,512] matmuls: per M-tile one
           stationary [128, 8] matrix holding each image's
           global-feature weights at that image's packed rows (zeros
           elsewhere), accumulated over the 7 M-tiles into one PSUM bank.
           Interleaved into the next pass's matmul stream (dep-anchored
           so the scheduler cannot hoist them ahead of the covering
           drain).  The last pass runs M-pair-grouped so its attention
           rows overlap the pass tail instead of serializing after it.

PSUM: 7 pass accumulators + 1 attention bank = exactly 8 banks.
"""
import time

import ml_dtypes
import numpy as np

import concourse.bacc as bacc
import concourse.mybir as mybir
import concourse.tile as tile
from concourse import bass_utils

B = 64
C = 100
LOOP = 3
FEAT = 2052
OUT = 2048
NCORES = 8
BPC = B // NCORES  # images per core
MROWS = BPC * C    # 800 packed M rows per core
TM = 128           # M-tile rows (128 enables the fast weight-load path)
NMT = 7            # M tiles of <=TM
NKT = 17           # K chunks: 16 x 128 + 1 x 5 (bbox + bias)
NNCH = 4           # N passes of 512

f32 = mybir.dt.float32
bf16 = mybir.dt.bfloat16
np_bf16 = ml_dtypes.bfloat16

_programs: dict = {}
last_results = None  # BassKernelResults of the most recent run (for harnesses)


def _occ_slots(key):
    """Occurrence index among equal-valued keys, stable order (matches jax ref)."""
    n = key.shape[0]
    order = np.argsort(key, kind="stable")
    sk = key[order]
    idx = np.arange(n)
    is_new = np.concatenate([[True], sk[1:] != sk[:-1]]) if n else np.zeros(0, bool)
    run_start = np.maximum.accumulate(np.where(is_new, idx, 0))
    pos = idx - run_start
    slots = np.zeros(n, np.int64)
    slots[order] = pos
    return slots


def _mw(m):
    return TM if m < NMT - 1 else MROWS - TM * (NMT - 1)


def _kw(k):
    return 128 if k < NKT - 1 else 5


def _build():
    nc = bacc.Bacc("TRN2", target_bir_lowering=False, debug=False,
                   num_devices=NCORES)

    xt_d = nc.dram_tensor("xt", [NKT, 128, MROWS], bf16, kind="ExternalInput").ap()
    gcwp_d = nc.dram_tensor("gcwp", [NNCH, NKT, 128, 512], bf16,
                            kind="ExternalInput").ap()
    gtp_d = nc.dram_tensor("gtp", [NMT, 128, BPC], bf16, kind="ExternalInput").ap()
    out_d = nc.dram_tensor("out", [BPC, OUT], f32, kind="ExternalOutput").ap()

    with tile.TileContext(nc) as tc:
        with tc.tile_pool(name="const", bufs=1) as cpool, \
             tc.tile_pool(name="sb", bufs=1) as pool, \
             tc.tile_pool(name="ps", bufs=1, space="PSUM") as psp:

            # gc_w resident: 68 chunk DMAs in phase-major order so phase 1's
            # chunks land first; gpsimd queue keeps sync free for xt
            gcw_sb = cpool.tile([128, NNCH * NKT * 512], bf16, tag="gcw")
            for nch in range(NNCH):
                for k in range(NKT):
                    kw = _kw(k)
                    o = (nch * NKT + k) * 512
                    nc.gpsimd.dma_start(gcw_sb[0:kw, o:o + 512],
                                        gcwp_d[nch][k][0:kw, :])
            # y^T chunks, k-major on the sync queue (consumed at ~1.5us/chunk)
            xt_sb = cpool.tile([128, NKT * MROWS], bf16, tag="xt")
            for k in range(NKT):
                kw = _kw(k)
                nc.sync.dma_start(xt_sb[0:kw, k * MROWS:(k + 1) * MROWS],
                                  xt_d[k][0:kw, :])
            # per-M-tile zero-padded attention weights
            gtp_sb = cpool.tile([128, NMT * BPC], bf16, tag="gtp")
            for m in range(NMT):
                nc.scalar.dma_start(gtp_sb[0:128, m * BPC:(m + 1) * BPC],
                                    gtp_d[m])
            # force the Lrelu spline-table load (~2.7us) under the DMA head
            warm = pool.tile([1, BPC], f32, tag="warm", bufs=1)
            nc.scalar.activation(warm[:], gtp_sb[0:1, 0:BPC],
                                 mybir.ActivationFunctionType.Lrelu, alpha=0.01)

            gcn = {}       # (nch, m) -> drained bf16 tile
            ps4 = {}       # nch -> attention PSUM tile
            ps4_mm = {}    # nch -> number of attention matmuls emitted

            def pass_mm(ps_tiles, nch, k, m):
                kw, mw = _kw(k), _mw(m)
                ko = k * MROWS + TM * m
                wo = (nch * NKT + k) * 512
                return nc.tensor.matmul(
                    ps_tiles[m][0:mw, 0:512],
                    xt_sb[0:kw, ko:ko + mw],
                    gcw_sb[0:kw, wo:wo + 512],
                    start=(k == 0), stop=(k == NKT - 1),
                )

            def drain(pst, nch, m, eng):
                # leaky-relu PSUM -> bf16; split across engines so the drains
                # of a tile clear inside the bank-reuse window
                mw = _mw(m)
                g = pool.tile([128, 512], bf16, tag="gcn", bufs=14,
                              name=f"gcn_{nch}_{m}")
                src = pst[0:mw, 0:512]
                if eng == "s":
                    nc.scalar.activation(g[0:mw, :], src,
                                         mybir.ActivationFunctionType.Lrelu,
                                         alpha=0.01)
                else:
                    tmp = pool.tile([128, 512], bf16, tag="lrt", bufs=2,
                                    name=f"lrt_{nch}_{m}")
                    nc.vector.tensor_scalar_mul(tmp[0:mw, :], src, 0.01)
                    nc.vector.tensor_max(g[0:mw, :], src, tmp[0:mw, :])
                gcn[(nch, m)] = g

            def ph4(nch, m, anchor=None):
                # attention row partials: [tw, 8]^T @ [tw, 512] accumulated
                # over the 7 M-tiles into one PSUM bank (rows 0:8)
                if nch not in ps4:
                    ps4[nch] = psp.tile([128, 512], f32, tag="ps4", bufs=1,
                                        name=f"ps4_{nch}")
                    ps4_mm[nch] = 0
                tw = _mw(m)
                mi = nc.tensor.matmul(
                    ps4[nch][0:BPC, 0:512],
                    gtp_sb[0:tw, m * BPC:(m + 1) * BPC],
                    gcn[(nch, m)][0:tw, 0:512],
                    start=(ps4_mm[nch] == 0), stop=(ps4_mm[nch] == NMT - 1),
                )
                ps4_mm[nch] += 1
                if anchor is not None:
                    tile.add_dep_helper(mi.ins, anchor.ins, sync=False,
                                        reason="defer ph4 behind pass")

            def ps4_out(nch):
                ost = pool.tile([BPC, 512], f32, tag="ost", bufs=2,
                                name=f"ost_{nch}")
                nc.vector.tensor_copy(ost[:], ps4[nch][0:BPC, 0:512])
                nc.sync.dma_start(out_d[0:BPC, nch * 512:(nch + 1) * 512],
                                  ost[:])

            # pass 0: K outer, M inner — consumes weight chunk k only ~1.5us
            # x k into the kernel, so the PE is never DMA-gated at the head.
            # Drains emitted bank-0/1 first (next pass needs those first).
            nch = 0
            ps_tiles = [psp.tile([128, 512], f32, tag=f"ps{m}", bufs=1,
                                 name=f"ps_{nch}_{m}") for m in range(NMT)]
            for k in range(NKT):
                for m in range(NMT):
                    pass_mm(ps_tiles, nch, k, m)
            for m, eng in ((0, "s"), (1, "s"), (2, "v"), (3, "v"),
                           (4, "s"), (5, "v"), (6, "s")):
                drain(ps_tiles[m], nch, m, eng)

            # passes 1..3: M-grouped (pairs + final triple, K interleaved
            # inside a group to keep same-bank accumulating matmuls apart).
            # Tile completions stagger through the pass, so drains run
            # mid-pass and every bank-reuse deadline has ~7us of slack.
            # The previous pass's attention rows interleave into groups 0-1;
            # pass 3's own interleave into its final group and tail.
            groups = [(0, 1), (2, 3), (4, 5, 6)]
            for nch in range(1, NNCH):
                ps_tiles = [psp.tile([128, 512], f32, tag=f"ps{m}", bufs=1,
                                     name=f"ps_{nch}_{m}") for m in range(NMT)]
                for gi, grp in enumerate(groups):
                    for k in range(NKT):
                        last = None
                        for m in grp:
                            last = pass_mm(ps_tiles, nch, k, m)
                        if gi == 0 and k in (4, 8, 12, 16):
                            ph4(nch - 1, (k - 4) // 4, anchor=last)
                        if gi == 1 and k in (4, 9, 14):
                            ph4(nch - 1, 4 + (k - 4) // 5, anchor=last)
                        if nch == NNCH - 1 and gi == 2 and k in (2, 6, 10, 14):
                            ph4(nch, (k - 2) // 4, anchor=last)
                    if gi == 1:
                        ps4_out(nch - 1)
                    for i, m in enumerate(grp):
                        drain(ps_tiles[m], nch, m, "s" if i % 2 == 0 else "v")
            nch = NNCH - 1
            for m in (4, 5, 6):
                ph4(nch, m)
            ps4_out(nch)

    nc.compile()
    return nc


def _get_program():
    if "main" not in _programs:
        _programs["main"] = _build()
    return _programs["main"]


def kernel(**inputs) -> np.ndarray:
    global last_results

    imf = np.asarray(inputs["image_features"], np.float32)
    bbox = np.asarray(inputs["bbox_list"], np.float32)
    gf = np.asarray(inputs["global_features"], np.float32)
    adj = np.asarray(inputs["adj"], np.float32)
    X = np.asarray(inputs["X"], np.float32)
    lin_w = np.asarray(inputs["lin_w"], np.float32)
    lin_b = np.float32(np.asarray(inputs["lin_b"]))
    gc_w = np.ascontiguousarray(np.asarray(inputs["gc_w"], np.float32))
    gc_b = np.asarray(inputs["gc_b"], np.float32)
    label = np.asarray(inputs["label_list"]).astype(np.int64)
    batch = np.asarray(inputs["batch"]).astype(np.int64)

    n = imf.shape[0]
    full = np.concatenate([imf, bbox], axis=1)

    # scatter bookkeeping, matching jax semantics: slots by stable order of
    # key=batch*C+(label-1); negative cats wrap, slot>=LOOP / far-oob dropped
    cat = label - 1
    key = batch * C + cat
    slots = _occ_slots(key)
    valid = (slots < LOOP) & (cat >= -C) & (cat < C)
    wvals = np.where(valid, lin_w[np.clip(slots, 0, LOOP - 1)], 0.0).astype(np.float32)
    cidx = np.mod(cat, C).astype(np.int64)

    # host scatter-sum: S[b,c,:] = sum of lin_w[slot]*full over the <=LOOP
    # boxes of bucket (b,c); slots are unique per bucket so per-slot
    # fancy-index adds have no collisions
    S = np.zeros((B, C, FEAT), np.float32)
    bok = valid & (batch >= -B) & (batch < B)
    bmod = np.mod(batch, B)
    for s in range(LOOP):
        sel = bok & (slots == s)
        if np.any(sel):
            S[bmod[sel], cidx[sel]] += wvals[sel, None] * full[sel]

    # pre-multiply the adjacency: y = (X + adj) @ (S + lin_b), f32 exact
    newadj = X[None, :, :] + adj                       # [B, C, C]
    y = np.matmul(newadj, S + lin_b)                   # [B, C, FEAT]

    # gc_w packed per (N-chunk, K-chunk); 17th chunk = bbox rows + gc_b row
    gcwp = np.zeros((NNCH, NKT, 128, 512), np.float32)
    gcwp[:, 0:16] = gc_w[0:2048].reshape(16, 128, NNCH, 512).transpose(2, 0, 1, 3)
    gcwp[:, 16, 0:4] = gc_w[2048:FEAT].reshape(4, NNCH, 512).transpose(1, 0, 2)
    gcwp[:, 16, 4] = gc_b.reshape(NNCH, 512)
    gcwp = gcwp.astype(np_bf16)

    in_maps = []
    for core in range(NCORES):
        imgs = slice(core * BPC, (core + 1) * BPC)
        yf = y[imgs].reshape(MROWS, FEAT)
        xt = np.zeros((NKT, 128, MROWS), np.float32)
        xt[0:16] = np.ascontiguousarray(yf[:, 0:2048].T).reshape(16, 128, MROWS)
        xt[16, 0:4] = yf[:, 2048:FEAT].T
        xt[16, 4] = 1.0
        # zero-padded attention weights: row r of M-tile m = packed row
        # R=TM*m+r = (image R//100, category R%100) -> gf value in column
        # R//100, zero elsewhere
        gtp = np.zeros((NMT, 128, BPC), np.float32)
        R = np.arange(MROWS)
        gtp[R // TM, R % TM, R // C] = gf[imgs][R // C, R % C]
        in_maps.append(dict(
            xt=xt.astype(np_bf16), gcwp=gcwp, gtp=gtp.astype(np_bf16)))

    nc = _get_program()
    res = None
    for attempt in range(4):
        try:
            res = bass_utils.run_bass_kernel_spmd(
                nc, in_maps, core_ids=list(range(NCORES)))
            break
        except Exception:
            if attempt == 3:
                raise
            time.sleep(3 * (attempt + 1))  # transient NRT exec-unit errors
    last_results = res
    return np.concatenate([res.results[i]["out"] for i in range(NCORES)], axis=0)
